# revision 1
# baseline (speedup 1.0000x reference)
"""Mistral sliding-window GQA attention + LoRA on 8 trn2 cores.

Sharding: DP2 x TP4. Core c -> batch b=c//4, head-slot s=c%4.
Each core: 8 q heads (2 kv groups of 4), full 2048-token sequence.
All matmuls fp32r (TF32-class, ~1e-4 rel err). Attention computed in
transposed layout (S^T tiles [k,q]), softmax without max subtraction
(scores are O(5)), denominators via ones-matmul, band masks generated
on host as 0/1 multiplicative tiles. Out-projection produces partial^T
[4096, 2048]; ReduceScatter(add) over each 4-core batch group splits
the output-channel axis; host transposes/concats.
"""
import math
from contextlib import ExitStack

import numpy as np

import concourse.bass as bass
import concourse.mybir as mybir
import concourse.tile as tile
from concourse import bacc
from concourse.bass_utils import run_bass_kernel_spmd
from concourse.masks import make_identity

F32 = mybir.dt.float32
F32R = mybir.dt.float32r
AF = mybir.ActivationFunctionType

HID = 4096
S = 2048
D = 128
WIN = 1024
NHQ = 8          # q heads per core
G = 2            # kv groups per core
HG = 4           # q heads per kv group
T = 512          # token chunk (matmul free dim)
NT = S // T      # 4
NHC = HID // 128  # 32 hidden chunks
NKT = S // 128    # 16 k tiles
LORA_R = 16
SCALE = 1.0 / math.sqrt(D)
LORA_SCALING = 2.0
EDGE_D0 = [-384, -256, -128, 0, 640, 768, 896, 1024]
EDGE_IDX = {d0: i for i, d0 in enumerate(EDGE_D0)}


def ktiles_for(q0):
    return [k0 for k0 in range(0, S, 128) if -384 <= q0 - k0 <= 1024]


_CACHE = {}
FLAGS = {"bcast": "gpsimd", "masks": True}


def build_nc(null=False, iters=1, upto="full"):
    key = ("null" if null else "full", iters, upto, tuple(sorted(FLAGS.items())))
    if key in _CACHE:
        return _CACHE[key]
    nc = bacc.Bacc("TRN2", target_bir_lowering=False, debug=False,
                   num_devices=8)
    d = {}
    for name, shape in [
        ("hst", [HID, S]), ("wq", [HID, 1024]), ("wk", [HID, 256]),
        ("wv", [HID, 256]), ("wo", [HID, 1024]), ("aq", [HID, LORA_R]),
        ("bq", [LORA_R, 1024]), ("av", [HID, LORA_R]),
        ("bv", [LORA_R, 256]), ("cost", [64, S]), ("sint", [64, S]),
        ("masks", [8, 128, T]),
    ]:
        d[name] = nc.dram_tensor(name, shape, F32, kind="ExternalInput").ap()
    out = nc.dram_tensor("out", [1024, S], F32, kind="ExternalOutput").ap()

    if null:
        _build_null(nc, d, out)
    else:
        _build_body(nc, d, out, iters, upto)
    nc.compile()
    _CACHE[key] = nc
    return nc


def _build_null(nc, d, out):
    with tile.TileContext(nc) as tc:
        with tc.tile_pool(name="sb", bufs=2) as sb:
            t = sb.tile([128, S], F32)
            nc.sync.dma_start(t[:], d["hst"][0:128, :])
            for i in range(8):
                nc.sync.dma_start(out[128 * i:128 * (i + 1), :], t[:])


def _build_body(nc, d, out, iters=1, upto="full"):
    with tile.TileContext(nc) as tc, ExitStack() as octx:
        cp = octx.enter_context(tc.tile_pool(name="const", bufs=1))
        dp = octx.enter_context(tc.tile_pool(name="dram", bufs=1, space="DRAM"))

        ident = cp.tile([128, 128], F32)
        make_identity(nc, ident[:])
        ones = cp.tile([128, 1], F32)
        nc.gpsimd.memset(ones[:], 1.0)
        ones_r = cp.tile([128, 1], F32R)
        nc.vector.tensor_copy(ones_r[:], ones[:])
        ones_row_f = cp.tile([1, 128], F32)
        nc.gpsimd.memset(ones_row_f[:], 1.0)
        ones_row = cp.tile([1, 128], F32R)
        nc.vector.tensor_copy(ones_row[:], ones_row_f[:])

        # LoRA weights: rounded residents (staging comes later via pst pool)
        aq_r = cp.tile([128, NHC, LORA_R], F32R)
        av_r = cp.tile([128, NHC, LORA_R], F32R)
        bq_r = cp.tile([LORA_R, 1024], F32R)
        bv_r = cp.tile([LORA_R, 256], F32R)

        attn_spill = dp.tile([NHQ, 128, S], F32)
        tm_dram = dp.tile([2, NT, LORA_R, T], F32)
        ag = [dp.tile([4 * HG, 128, S], F32, name=f"ag{g}") for g in range(G)]

        for rep in range(iters):
          _one_rep(nc, tc, d, out, rep, ident, ones_r, ones_row, aq_r, av_r,
                   bq_r, bv_r, attn_spill, tm_dram, ag, upto)


def _one_rep(nc, tc, d, out, rep, ident, ones_r, ones_row, aq_r, av_r,
             bq_r, bv_r, attn_spill, tm_dram, ag, upto="full"):
        pctx = ExitStack()
        pa = pctx.enter_context(tc.tile_pool(name=f"pa{rep}", bufs=1))
        pst = pctx.enter_context(tc.tile_pool(name=f"pstream{rep}", bufs=1))

        if rep == 0:
            # stage + round lora weights through stream tags
            aq_st = pst.tile([128, NHC, LORA_R], F32, tag="hst", bufs=2)
            nc.sync.dma_start(aq_st[:],
                              d["aq"].rearrange("(c p) r -> p c r", p=128))
            nc.vector.tensor_copy(aq_r[:], aq_st[:])
            av_st = pst.tile([128, NHC, LORA_R], F32, tag="hst", bufs=2)
            nc.sync.dma_start(av_st[:],
                              d["av"].rearrange("(c p) r -> p c r", p=128))
            nc.vector.tensor_copy(av_r[:], av_st[:])
            for half in range(2):
                bq_st = pst.tile([LORA_R, T], F32, tag="tms2", bufs=2,
                                 name=f"bqst{half}")
                nc.sync.dma_start(bq_st[:], d["bq"][:, T * half:T * (half + 1)])
                nc.vector.tensor_copy(bq_r[:, T * half:T * (half + 1)], bq_st[:])
            bv_st = pst.tile([LORA_R, T], F32, tag="tms2", bufs=2)
            nc.sync.dma_start(bv_st[0:LORA_R, 0:256], d["bv"][:])
            nc.vector.tensor_copy(bv_r[:], bv_st[0:LORA_R, 0:256])

        qtg = pa.tile([128, HG, S], F32R, tag="qtg")
        ktg = pa.tile([128, S], F32R, tag="ktg")
        vng = pa.tile([128, NKT, 128], F32R, tag="vng")

        def rope_into(ps, cs, sn, dst):
            # dst = ps*cos + rotate_half(ps)*sin, written as f32r
            c1 = pst.tile([128, T], F32, tag="rpc")
            nc.vector.tensor_mul(c1[0:64, :], ps[0:64, :], cs[:])
            nc.vector.tensor_mul(c1[64:128, :], ps[64:128, :], cs[:])
            s1 = pst.tile([128, T], F32, tag="rps")
            nc.vector.tensor_mul(s1[0:64, :], ps[64:128, :], sn[:])
            nc.vector.tensor_mul(s1[64:128, :], ps[0:64, :], sn[:])
            nc.vector.tensor_sub(dst[0:64, :], c1[0:64, :], s1[0:64, :])
            nc.vector.tensor_add(dst[64:128, :], c1[64:128, :], s1[64:128, :])

        for g in range(G):
            # ---------------- projection phase for group g ----------------
            with tc.tile_pool(name=f"w{g}_{rep}", bufs=1) as wp, \
                 tc.tile_pool(name=f"pps{g}_{rep}", bufs=1, space="PSUM") as pps:
                wq_r = wp.tile([128, NHC, 512], F32R, tag="wqr")
                wk_r = wp.tile([128, NHC, 128], F32R, tag="wkr")
                wv_r = wp.tile([128, NHC, 128], F32R, tag="wvr")
                for hc in range(NHC):
                    st = pst.tile([128, 512], F32, tag="wst", bufs=2)
                    nc.sync.dma_start(
                        st[:], d["wq"][128 * hc:128 * (hc + 1),
                                       512 * g:512 * (g + 1)])
                    nc.vector.tensor_copy(wq_r[:, hc, :], st[:])
                    stk = pst.tile([128, 256], F32, tag="wkst", bufs=2)
                    nc.sync.dma_start(
                        stk[:, 0:128], d["wk"][128 * hc:128 * (hc + 1),
                                               128 * g:128 * (g + 1)])
                    nc.sync.dma_start(
                        stk[:, 128:256], d["wv"][128 * hc:128 * (hc + 1),
                                                 128 * g:128 * (g + 1)])
                    nc.vector.tensor_copy(wk_r[:, hc, :], stk[:, 0:128])
                    nc.vector.tensor_copy(wv_r[:, hc, :], stk[:, 128:256])

                for t in range(NT):
                    q0 = t * T
                    qps = [pps.tile([128, T], F32, tag=f"q{i}", name=f"qps{i}")
                           for i in range(HG)]
                    kps = pps.tile([128, T], F32, tag="k")
                    vps = pps.tile([128, T], F32, tag="v")
                    if g == 0:
                        lpq = pps.tile([LORA_R, T], F32, tag="lpq")
                        lpv = pps.tile([LORA_R, T], F32, tag="lpv")
                    for hc in range(NHC):
                        hst_st = pst.tile([128, T], F32, tag="hst", bufs=2)
                        nc.sync.dma_start(
                            hst_st[:], d["hst"][128 * hc:128 * (hc + 1),
                                                q0:q0 + T])
                        hst_r = pst.tile([128, T], F32R, tag="hsr", bufs=2)
                        nc.scalar.copy(hst_r[:], hst_st[:])
                        for i in range(HG):
                            nc.tensor.matmul(
                                qps[i][:], wq_r[:, hc, 128 * i:128 * (i + 1)],
                                hst_r[:], start=(hc == 0), stop=False)
                        nc.tensor.matmul(kps[:], wk_r[:, hc, :], hst_r[:],
                                         start=(hc == 0), stop=(hc == NHC - 1))
                        nc.tensor.matmul(vps[:], wv_r[:, hc, :], hst_r[:],
                                         start=(hc == 0), stop=False)
                        if g == 0:
                            nc.tensor.matmul(lpq[:], aq_r[:, hc, :], hst_r[:],
                                             start=(hc == 0),
                                             stop=(hc == NHC - 1))
                            nc.tensor.matmul(lpv[:], av_r[:, hc, :], hst_r[:],
                                             start=(hc == 0),
                                             stop=(hc == NHC - 1))
                    if g == 0:
                        tmq_sb = pst.tile([LORA_R, T], F32R, tag="tms", bufs=2)
                        nc.vector.tensor_copy(tmq_sb[:], lpq[:])
                        nc.sync.dma_start(tm_dram[0, t], tmq_sb[:].bitcast(F32))
                        tmv_sb = pst.tile([LORA_R, T], F32R, tag="tms", bufs=2)
                        nc.vector.tensor_copy(tmv_sb[:], lpv[:])
                        nc.sync.dma_start(tm_dram[1, t], tmv_sb[:].bitcast(F32))
                    else:
                        tmq_st = pst.tile([LORA_R, T], F32, tag="tms2", bufs=2)
                        nc.sync.dma_start(tmq_st[:], tm_dram[0, t])
                        tmq_sb = pst.tile([LORA_R, T], F32R, tag="tms", bufs=2)
                        nc.vector.tensor_copy(tmq_sb[:], tmq_st[:])
                        tmv_st = pst.tile([LORA_R, T], F32, tag="tms2", bufs=2)
                        nc.sync.dma_start(tmv_st[:], tm_dram[1, t])
                        tmv_sb = pst.tile([LORA_R, T], F32R, tag="tms", bufs=2)
                        nc.vector.tensor_copy(tmv_sb[:], tmv_st[:])
                    # LoRA second stage accumulates into the open psum groups
                    for i in range(HG):
                        hg = g * HG + i
                        nc.tensor.matmul(
                            qps[i][:], bq_r[:, 128 * hg:128 * (hg + 1)],
                            tmq_sb[:], start=False, stop=True)
                    nc.tensor.matmul(vps[:], bv_r[:, 128 * g:128 * (g + 1)],
                                     tmv_sb[:], start=False, stop=True)
                    # epilogues: RoPE for q/k, transpose for v
                    cs = pst.tile([64, T], F32, tag="cost", bufs=2)
                    nc.sync.dma_start(cs[:], d["cost"][:, q0:q0 + T])
                    sn = pst.tile([64, T], F32, tag="sint", bufs=2)
                    nc.sync.dma_start(sn[:], d["sint"][:, q0:q0 + T])
                    for i in range(HG):
                        rope_into(qps[i], cs, sn, qtg[:, i, q0:q0 + T])
                    rope_into(kps, cs, sn, ktg[:, q0:q0 + T])
                    vev = pst.tile([128, T], F32, tag="vev", bufs=1)
                    nc.vector.tensor_copy(vev[:], vps[:])
                    for tt in range(4):
                        vtp = pps.tile([128, 128], F32, tag="lpv")
                        nc.tensor.transpose(
                            vtp[:], vev[:, 128 * tt:128 * (tt + 1)], ident[:])
                        nc.vector.tensor_copy(vng[:, 4 * t + tt, :], vtp[:])

            # ---------------- attention phase for group g ----------------
            if upto == "proj":
                continue
            with tc.tile_pool(name=f"am{g}_{rep}", bufs=1) as amp, \
                 tc.tile_pool(name=f"aps{g}_{rep}", bufs=1, space="PSUM") as aps:
                for i in range(HG):
                    hg = g * HG + i
                    for qc in range(NT):
                        q0 = qc * T
                        kts = ktiles_for(q0)
                        avp = aps.tile([128, T], F32, tag="avps", bufs=2)
                        dnp = aps.tile([1, T], F32, tag="dps", bufs=1)
                        last = len(kts) - 1
                        for ki, k0 in enumerate(kts):
                            sps = aps.tile([128, T], F32, tag="sps", bufs=4)
                            nc.tensor.matmul(
                                sps[:], ktg[:, k0:k0 + 128],
                                qtg[:, i, q0:q0 + T], start=True, stop=True)
                            d0 = q0 - k0
                            at = amp.tile([128, T], F32R, tag="at", bufs=3)
                            nc.scalar.activation(at[:], sps[:], AF.Exp)
                            if d0 in EDGE_IDX and FLAGS["masks"]:
                                # zero where (qq - kk + d0) < 0  (causal)
                                if d0 - 127 < 0:
                                    nc.gpsimd.affine_select(
                                        out=at[:], in_=at[:],
                                        pattern=[[1, T]],
                                        compare_op=mybir.AluOpType.is_ge,
                                        fill=0.0, base=d0,
                                        channel_multiplier=-1)
                                # zero where (qq - kk + d0) > 1023 (window)
                                if d0 + T - 1 > 1023:
                                    nc.gpsimd.affine_select(
                                        out=at[:], in_=at[:],
                                        pattern=[[-1, T]],
                                        compare_op=mybir.AluOpType.is_ge,
                                        fill=0.0, base=1023 - d0,
                                        channel_multiplier=1)
                            nc.tensor.matmul(avp[:], vng[:, k0 // 128, :],
                                             at[:], start=(ki == 0),
                                             stop=(ki == last))
                            nc.tensor.matmul(dnp[:], ones_r[:], at[:],
                                             start=(ki == 0), stop=(ki == last))
                        if FLAGS["bcast"] == "gpsimd":
                            rc = amp.tile([1, T], F32, tag="rc", bufs=1)
                            nc.vector.reciprocal(rc[:], dnp[:])
                            bc = amp.tile([128, T], F32, tag="bc", bufs=2)
                            nc.gpsimd.partition_broadcast(bc[:], rc[:])
                        else:
                            rc = amp.tile([1, T], F32R, tag="rc", bufs=1)
                            with nc.allow_low_precision(reason="fp32r round"):
                                nc.vector.reciprocal(rc[:], dnp[:])
                            bcp = aps.tile([128, T], F32, tag="bcp", bufs=1)
                            nc.tensor.matmul(bcp[:], ones_row[:], rc[:],
                                             start=True, stop=True)
                            bc = amp.tile([128, T], F32, tag="bc", bufs=2)
                            nc.scalar.copy(bc[:], bcp[:])
                        ao = amp.tile([128, T], F32R, tag="ao", bufs=2)
                        nc.vector.tensor_mul(ao[:], avp[:], bc[:])
                        nc.sync.dma_start(attn_spill[hg, :, q0:q0 + T],
                                          ao[:].bitcast(F32))
                if upto == "full":
                    nc.gpsimd.collective_compute(
                        "AllGather", mybir.AluOpType.bypass,
                        replica_groups=[[0, 1, 2, 3], [4, 5, 6, 7]],
                        ins=[attn_spill[HG * g:HG * (g + 1)].opt()],
                        outs=[ag[g].opt()])

        pctx.close()

        # ---------------- output projection (local column slice) ----------------
        with tc.tile_pool(name=f"op{rep}", bufs=1) as op, \
             tc.tile_pool(name=f"ost{rep}", bufs=1) as ost, \
             tc.tile_pool(name=f"ops{rep}", bufs=1, space="PSUM") as opsp:
            wo_r = op.tile([128, 32, 8, 128], F32R)
            for dc in range(32):
                st = ost.tile([128, 1024], F32, tag="wost", bufs=2)
                nc.sync.dma_start(
                    st[:], d["wo"][128 * dc:128 * (dc + 1), :])
                dstv = wo_r[:, dc, :, :].rearrange("p a b -> p (a b)")
                nc.vector.tensor_copy(dstv, st[:])
            # head H (global contraction chunk) -> (src half, ag row)
            def src_of(H):
                return (H % 8) // 4, 4 * (H // 8) + (H % 4)
            halves = [[H for H in range(32) if (H % 8) // 4 == h]
                      for h in range(2)]
            for tt in range(NT):
                ts0 = tt * T
                psums = [opsp.tile([128, T], F32, tag=f"o{oc}", name=f"ops{oc}")
                         for oc in range(8)]
                for half in range(2):
                    atr = {}
                    for j, H in enumerate(halves[half]):
                        ast = ost.tile([128, T], F32, tag=f"ast{j % 4}",
                                       bufs=2, name=f"ast{j}")
                        g_src, row = src_of(H)
                        nc.sync.dma_start(ast[:], ag[g_src][row, :, ts0:ts0 + T])
                        ar = ost.tile([128, T], F32R, tag=f"atr{j}",
                                      name=f"atr{j}")
                        nc.scalar.copy(ar[:], ast[:])
                        atr[H] = ar
                    for oc in range(8):
                        for jj, H in enumerate(halves[half]):
                            nc.tensor.matmul(
                                psums[oc][:], wo_r[:, H, oc, :], atr[H][:],
                                start=(half == 0 and jj == 0),
                                stop=(half == 1 and jj == 15))
                for oc in range(8):
                    ev = ost.tile([128, T], F32, tag="oev", bufs=3,
                                  name=f"ev{oc}")
                    nc.scalar.copy(ev[:], psums[oc][:])
                    nc.sync.dma_start(
                        out[128 * oc:128 * (oc + 1), ts0:ts0 + T], ev[:])


def prep_inputs(inputs):
    hs = np.asarray(inputs["hidden_states"], dtype=np.float32)
    pos = np.asarray(inputs["position_ids"]).astype(np.float64)
    Wq = np.asarray(inputs["Wq"], dtype=np.float32)
    Wk = np.asarray(inputs["Wk"], dtype=np.float32)
    Wv = np.asarray(inputs["Wv"], dtype=np.float32)
    Wo = np.asarray(inputs["Wo"], dtype=np.float32)
    aq = np.asarray(inputs["lora_A_q"], dtype=np.float32)
    bq = np.asarray(inputs["lora_B_q"], dtype=np.float32)
    av = np.asarray(inputs["lora_A_v"], dtype=np.float32)
    bv = np.asarray(inputs["lora_B_v"], dtype=np.float32)

    wq_eff = (Wq * SCALE).astype(np.float32)
    bq_eff = (bq * (LORA_SCALING * SCALE)).astype(np.float32)
    bv_eff = (bv * LORA_SCALING).astype(np.float32)

    # RoPE tables per batch, transposed to [d/2, S]
    inv_freq = 1.0 / (10000.0 ** (np.arange(0, D, 2, dtype=np.float64) / D))
    tabs = []
    for b in range(2):
        freqs = np.outer(pos[b], inv_freq)          # [S, 64]
        tabs.append((np.ascontiguousarray(np.cos(freqs).T.astype(np.float32)),
                     np.ascontiguousarray(np.sin(freqs).T.astype(np.float32))))
    hsT = [np.ascontiguousarray(hs[b].T) for b in range(2)]

    # 0/1 edge mask tiles [8, 128, T]
    masks = np.zeros((8, 128, T), dtype=np.float32)
    kk = np.arange(128)[:, None]
    qq = np.arange(T)[None, :]
    for idx, d0 in enumerate(EDGE_D0):
        dd = d0 + qq - kk
        masks[idx] = ((dd >= 0) & (dd < WIN)).astype(np.float32)

    in_maps = []
    for c in range(8):
        b, s = divmod(c, 4)
        cos_b, sin_b = tabs[b]
        in_maps.append({
            "hst": hsT[b],
            "wq": np.ascontiguousarray(wq_eff[:, 1024 * s:1024 * (s + 1)]),
            "wk": np.ascontiguousarray(Wk[:, 256 * s:256 * (s + 1)]),
            "wv": np.ascontiguousarray(Wv[:, 256 * s:256 * (s + 1)]),
            "wo": np.ascontiguousarray(Wo[:, 1024 * s:1024 * (s + 1)]),
            "aq": aq, "av": av,
            "bq": np.ascontiguousarray(bq_eff[:, 1024 * s:1024 * (s + 1)]),
            "bv": np.ascontiguousarray(bv_eff[:, 256 * s:256 * (s + 1)]),
            "cost": cos_b, "sint": sin_b, "masks": masks,
        })
    return in_maps


def assemble(results):
    out = np.empty((2, S, HID), dtype=np.float32)
    for c in range(8):
        b, r = divmod(c, 4)
        out[b, :, 1024 * r:1024 * (r + 1)] = results[c]["out"].T
    return out


def run_prepped(in_maps, null=False, iters=1):
    nc = build_nc(null=null, iters=iters)
    return run_bass_kernel_spmd(nc, in_maps, list(range(8)), trace=False)


def kernel(**inputs) -> np.ndarray:
    in_maps = prep_inputs(inputs)
    res = run_prepped(in_maps)
    return assemble(res.results)



# revision 23
# speedup vs baseline: 3.4666x; 3.4666x over previous
"""Mistral sliding-window GQA attention + LoRA on 8 trn2 cores.

Sharding: DP2 x TP4. Core c -> batch b=c//4, head-slot s=c%4.
Each core: 8 q heads (2 kv groups of 4), full 2048-token sequence.

bf16 datapath (weights + activations shipped as bf16; PSUM accumulation
fp32). Hidden states are streamed once; q/k/v/LoRA projections for both
kv groups run in a single For_i loop over 512-token chunks. Attention
in transposed layout (S^T tiles [k,q]) with softmax denominators via
ones-matmul and band masks via affine_select; one For_i loop over the
4 heads of each kv group. AllGather(bf16) shares attention outputs
across each 4-core batch group; the output projection is a For_i loop
over 512-token chunks. Hardware loops + batched rearranged DMAs keep
the emitted instruction count (and so NEFF size / load time) small.
"""
import math
from contextlib import ExitStack

import ml_dtypes
import numpy as np

import concourse.bass as bass
import concourse.mybir as mybir
import concourse.tile as tile
from concourse import bacc
from concourse.bass import ds
from concourse.bass_utils import run_bass_kernel_spmd
from concourse.masks import make_identity

F32 = mybir.dt.float32
BF16 = mybir.dt.bfloat16
AF = mybir.ActivationFunctionType
BF = ml_dtypes.bfloat16

HID = 4096
S = 2048
D = 128
WIN = 1024
NHQ = 8          # q heads per core
G = 2            # kv groups per core
HG = 4           # q heads per kv group
T = 512          # token chunk (matmul free dim)
NT = S // T      # 4
NHC = HID // 128  # 32 hidden chunks
LORA_R = 16
SCALE = 1.0 / math.sqrt(D)
LORA_SCALING = 2.0


def ktiles_for(q0):
    return [k0 for k0 in range(0, S, 128) if -384 <= q0 - k0 <= 1024]


_CACHE = {}


def build_nc(null=False, iters=1, upto="full", dump=False):
    key = ("null" if null else "full", iters, upto, dump)
    if key in _CACHE:
        return _CACHE[key]
    nc = bacc.Bacc("TRN2", target_bir_lowering=False, debug=False,
                   num_devices=8)
    d = {}
    for name, shape, dt in [
        ("hstq", [HID // 4, S], BF16), ("wq", [HID, 1024], BF16),
        ("wk", [HID, 256], BF16), ("wv", [HID, 256], BF16),
        ("wo", [HID, 1024], BF16), ("aq", [HID, LORA_R], BF16),
        ("bq", [LORA_R, 1024], BF16), ("av", [HID, LORA_R], BF16),
        ("bv", [LORA_R, 256], BF16), ("cst", [128, S], F32),
    ]:
        d[name] = nc.dram_tensor(name, shape, dt, kind="ExternalInput").ap()
    out = nc.dram_tensor("out", [1024, S], F32, kind="ExternalOutput").ap()
    if dump:
        for name, shape in [("qtg_d", [128, NHQ, S]), ("ktg_d", [128, G, S]),
                            ("vng_d", [128, G, S]),
                            ("as_d", [NHQ, 128, S])]:
            d[name] = nc.dram_tensor(name, shape, BF16,
                                     kind="ExternalOutput").ap()

    if null:
        _build_null(nc, d, out)
    else:
        _build_body(nc, d, out, iters, upto, dump)
    nc.compile()
    _CACHE[key] = nc
    return nc


def _build_null(nc, d, out):
    with tile.TileContext(nc) as tc:
        with tc.tile_pool(name="sb", bufs=2) as sb:
            t = sb.tile([128, S], F32)
            nc.gpsimd.memset(t[:], 0.0)
            for i in range(8):
                nc.sync.dma_start(out[128 * i:128 * (i + 1), :], t[:])


def _build_body(nc, d, out, iters=1, upto="full", dump=False):
    with tile.TileContext(nc) as tc, ExitStack() as octx:
        cp = octx.enter_context(tc.tile_pool(name="const", bufs=1))
        dp = octx.enter_context(tc.tile_pool(name="dram", bufs=1, space="DRAM"))

        ident = cp.tile([128, 128], F32)
        make_identity(nc, ident[:])
        ones_b = cp.tile([128, 1], BF16)
        nc.gpsimd.memset(ones_b[:], 1.0)

        attn_spill = dp.tile([NHQ, 128, NT, T], BF16)
        ag = [dp.tile([4 * HG, 128, S], BF16, name=f"ag{g}") for g in range(G)]

        # gather the full hidden-state transpose from per-core quarters
        # (collectives may not read IO tensors -> bounce through a dram tile)
        hst_q = dp.tile([HID // 4, S], BF16, name="hstb")
        nc.sync.dma_start(hst_q[:], d["hstq"][:])
        hst_full = dp.tile([HID, S], BF16, name="hstf")
        nc.gpsimd.collective_compute(
            "AllGather", mybir.AluOpType.bypass,
            replica_groups=[[0, 1, 2, 3], [4, 5, 6, 7]],
            ins=[hst_q.opt()], outs=[hst_full.opt()])
        d = dict(d, hst=hst_full[:])

        for rep in range(iters):
            _one_rep(nc, tc, d, out, rep, ident, ones_b, attn_spill, ag, upto,
                     dump)


def _one_rep(nc, tc, d, out, rep, ident, ones_b, attn_spill, ag, upto="full",
             dump=False):
    pctx = ExitStack()
    ap_ = pctx.enter_context(tc.tile_pool(name=f"act{rep}", bufs=1))

    # ---- attention operands (filled by projection loop) ----
    qtg = ap_.tile([128, NHQ, NT, T], BF16)
    ktg = ap_.tile([128, G, NT, T], BF16)
    vng = ap_.tile([128, G, NT, T], BF16)   # v^T: [k-part, t, d-cols]

    hst_v = d["hst"].rearrange("(c p) (t s) -> p c t s", p=128, t=NT)

    def rope_into(ps, cs, sn, dst, tmp):
        # dst = ps*cos + rotate_half(ps)*sin  (halves along partition dim)
        c1 = tmp.tile([128, T], F32, tag="rpc", bufs=1)
        nc.vector.tensor_mul(c1[0:64, :], ps[0:64, :], cs)
        nc.vector.tensor_mul(c1[64:128, :], ps[64:128, :], cs)
        s1 = tmp.tile([128, T], F32, tag="rps", bufs=1)
        nc.vector.tensor_mul(s1[0:64, :], ps[64:128, :], sn)
        nc.vector.tensor_mul(s1[64:128, :], ps[0:64, :], sn)
        ro = tmp.tile([128, T], BF16, tag="rpo", bufs=2)
        nc.vector.tensor_sub(ro[0:64, :], c1[0:64, :], s1[0:64, :])
        nc.vector.tensor_add(ro[64:128, :], c1[64:128, :], s1[64:128, :])
        nc.vector.tensor_copy(dst, ro[:])

    # =================== projection phase ===================
    with tc.tile_pool(name=f"w{rep}", bufs=1) as wp, \
         tc.tile_pool(name=f"pst{rep}", bufs=1) as pst, \
         tc.tile_pool(name=f"pps{rep}", bufs=1, space="PSUM") as pps:
        # resident weights (single rearranged DMAs)
        wq_r = wp.tile([128, NHC, 1024], BF16)
        nc.sync.dma_start(wq_r[:], d["wq"].rearrange("(c p) n -> p c n", p=128))
        wk_r = wp.tile([128, NHC, 256], BF16)
        nc.sync.dma_start(wk_r[:], d["wk"].rearrange("(c p) n -> p c n", p=128))
        wv_r = wp.tile([128, NHC, 256], BF16)
        nc.sync.dma_start(wv_r[:], d["wv"].rearrange("(c p) n -> p c n", p=128))
        aq_r = wp.tile([128, NHC, LORA_R], BF16)
        nc.sync.dma_start(aq_r[:], d["aq"].rearrange("(c p) r -> p c r", p=128))
        av_r = wp.tile([128, NHC, LORA_R], BF16)
        nc.sync.dma_start(av_r[:], d["av"].rearrange("(c p) r -> p c r", p=128))
        bq_r = wp.tile([LORA_R, 1024], BF16)
        nc.sync.dma_start(bq_r[:], d["bq"][:])
        bv_r = wp.tile([LORA_R, 256], BF16)
        nc.sync.dma_start(bv_r[:], d["bv"][:])
        cst_v = d["cst"].rearrange("p (t s) -> p t s", t=NT)

        with tc.For_i(0, NT, 1, name=f"proj{rep}") as t:
            hst_r = pst.tile([128, NHC, T], BF16, tag="hst", bufs=1)
            nc.sync.dma_start(hst_r[:], hst_v[:, :, ds(t, 1), :])
            cst_c = pst.tile([128, T], F32, tag="cst", bufs=1)
            nc.sync.dma_start(cst_c[:], cst_v[:, ds(t, 1), :])
            cs = cst_c[0:64, :]
            sn = cst_c[64:128, :]
            qps = [pps.tile([128, T], F32, tag=f"q{i}", name=f"qps{i}")
                   for i in range(HG)]
            kps = pps.tile([128, T], F32, tag="k")
            vps = pps.tile([128, T], F32, tag="v")
            lpq = pps.tile([LORA_R, T], F32, tag="lq")
            lpv = pps.tile([LORA_R, T], F32, tag="lv")
            tmq = pst.tile([LORA_R, T], BF16, tag="tmq", bufs=1)
            tmv = pst.tile([LORA_R, T], BF16, tag="tmv", bufs=1)
            for g in range(G):
                for hc in range(NHC):
                    h = hst_r[:, hc, :]
                    for i in range(HG):
                        c0 = 512 * g + 128 * i
                        nc.tensor.matmul(qps[i][:], wq_r[:, hc, c0:c0 + 128],
                                         h, start=(hc == 0), stop=False)
                    nc.tensor.matmul(kps[:], wk_r[:, hc, 128 * g:128 * (g + 1)],
                                     h, start=(hc == 0), stop=(hc == NHC - 1))
                    nc.tensor.matmul(vps[:], wv_r[:, hc, 128 * g:128 * (g + 1)],
                                     h, start=(hc == 0), stop=False)
                    if g == 0:
                        nc.tensor.matmul(lpq[:], aq_r[:, hc, :], h,
                                         start=(hc == 0), stop=(hc == NHC - 1))
                        nc.tensor.matmul(lpv[:], av_r[:, hc, :], h,
                                         start=(hc == 0), stop=(hc == NHC - 1))
                if g == 0:
                    nc.vector.tensor_copy(tmq[:], lpq[:])
                    nc.vector.tensor_copy(tmv[:], lpv[:])
                # LoRA second stage closes the accumulation groups
                for i in range(HG):
                    c0 = 512 * g + 128 * i
                    nc.tensor.matmul(qps[i][:], bq_r[:, c0:c0 + 128], tmq[:],
                                     start=False, stop=True)
                nc.tensor.matmul(vps[:], bv_r[:, 128 * g:128 * (g + 1)],
                                 tmv[:], start=False, stop=True)
                # epilogues: RoPE q/k, transpose v
                for i in range(HG):
                    rope_into(qps[i], cs, sn, qtg[:, g * HG + i, ds(t, 1), :],
                              pst)
                rope_into(kps, cs, sn, ktg[:, g, ds(t, 1), :], pst)
                vev = pst.tile([128, T], F32, tag="vev", bufs=1)
                nc.vector.tensor_copy(vev[:], vps[:])
                for tt in range(4):
                    vtp = pps.tile([128, 128], F32, tag="lq" if tt % 2 == 0
                                   else "lv", name=f"vtp{tt}")
                    nc.tensor.transpose(vtp[:], vev[:, 128 * tt:128 * (tt + 1)],
                                        ident[:])
                    nc.vector.tensor_copy(
                        vng[:, g, ds(t, 1), 128 * tt:128 * (tt + 1)], vtp[:])

    if dump:
        nc.sync.dma_start(d["qtg_d"][:],
                          qtg[:].rearrange("p h t s -> p h (t s)"))
        nc.sync.dma_start(d["ktg_d"][:],
                          ktg[:].rearrange("p g t s -> p g (t s)"))
        nc.sync.dma_start(d["vng_d"][:],
                          vng[:].rearrange("p g t s -> p g (t s)"))
    if upto == "proj":
        pctx.close()
        return

    # wo resident load overlaps the attention phase
    op = pctx.enter_context(tc.tile_pool(name=f"wo{rep}", bufs=1))
    wo_r = op.tile([128, NHC, 8, 128], BF16)
    nc.sync.dma_start(
        wo_r[:], d["wo"].rearrange("(c p) (o q) -> p c o q", p=128, o=8))

    # =================== attention phase ===================
    ktg_f = [ktg[:, g].rearrange("p t s -> p (t s)") for g in range(G)]
    vng_f = [vng[:, g].rearrange("p t s -> p (t s)") for g in range(G)]
    with tc.tile_pool(name=f"am{rep}", bufs=1) as amp, \
         tc.tile_pool(name=f"aps{rep}", bufs=1, space="PSUM") as aps:
        for g in range(G):
            with tc.For_i(g * HG, (g + 1) * HG, 1, name=f"attn{g}_{rep}") as hh:
                for qc in range(NT):
                    q0 = qc * T
                    kts = ktiles_for(q0)
                    avp = aps.tile([128, T], F32, tag="avps", bufs=2)
                    dnp = aps.tile([1, T], F32, tag="dps", bufs=2)
                    last = len(kts) - 1
                    for ki, k0 in enumerate(kts):
                        sps = aps.tile([128, T], F32, tag="sps", bufs=2)
                        nc.tensor.matmul(
                            sps[:], ktg_f[g][:, k0:k0 + 128],
                            qtg[:, ds(hh, 1), qc, :], start=True, stop=True)
                        d0 = q0 - k0
                        at = amp.tile([128, T], BF16, tag="at", bufs=3)
                        nc.scalar.activation(at[:], sps[:], AF.Exp)
                        if d0 - 127 < 0:
                            # zero where (qq - kk + d0) < 0  (causal)
                            nc.gpsimd.affine_select(
                                out=at[:], in_=at[:], pattern=[[1, T]],
                                compare_op=mybir.AluOpType.is_ge,
                                fill=0.0, base=d0, channel_multiplier=-1)
                        if d0 + T - 1 > WIN - 1:
                            # zero where (qq - kk + d0) > WIN-1 (window)
                            nc.gpsimd.affine_select(
                                out=at[:], in_=at[:], pattern=[[-1, T]],
                                compare_op=mybir.AluOpType.is_ge,
                                fill=0.0, base=WIN - 1 - d0,
                                channel_multiplier=1)
                        nc.tensor.matmul(avp[:], vng_f[g][:, k0:k0 + 128],
                                         at[:], start=(ki == 0),
                                         stop=(ki == last))
                        nc.tensor.matmul(dnp[:], ones_b[:], at[:],
                                         start=(ki == 0), stop=(ki == last))
                    rc = amp.tile([1, T], F32, tag="rc", bufs=2)
                    nc.vector.reciprocal(rc[:], dnp[:])
                    bc = amp.tile([128, T], F32, tag="bc", bufs=2)
                    nc.gpsimd.partition_broadcast(bc[:], rc[:])
                    ao = amp.tile([128, T], BF16, tag="ao", bufs=2)
                    nc.vector.tensor_mul(ao[:], avp[:], bc[:])
                    nc.sync.dma_start(attn_spill[ds(hh, 1), :, qc, :], ao[:])
        if dump:
            with tc.tile_pool(name=f"dmp{rep}", bufs=1) as dmp:
                for h in range(NHQ):
                    dt_ = dmp.tile([128, S], BF16, tag="dd", bufs=2)
                    nc.sync.dma_start(
                        dt_[:],
                        attn_spill[h].rearrange("p t s -> p (t s)"))
                    nc.sync.dma_start(d["as_d"][h], dt_[:])
        if upto == "full":
            for g in range(G):
                nc.gpsimd.collective_compute(
                    "AllGather", mybir.AluOpType.bypass,
                    replica_groups=[[0, 1, 2, 3], [4, 5, 6, 7]],
                    ins=[attn_spill[HG * g:HG * (g + 1)].opt()],
                    outs=[ag[g].opt()])

    # =================== output projection ===================
    # global head H lives at ag[(H % 8) // 4][4 * (H // 8) + H % 4]
    ag_v = [ag[g].rearrange("r p s -> p r s") for g in range(G)]
    out_v = out.rearrange("(o p) s -> p o s", p=128)
    with tc.tile_pool(name=f"ost{rep}", bufs=1) as ost, \
         tc.tile_pool(name=f"ops{rep}", bufs=1, space="PSUM") as opsp:
        with tc.For_i(0, S, T, name=f"oproj{rep}") as ts0:
            asts = []
            for gs in range(G):
                ast = ost.tile([128, 4 * HG, T], BF16, tag=f"ast{gs}", bufs=1)
                nc.sync.dma_start(ast[:], ag_v[gs][:, :, ds(ts0, T)])
                asts.append(ast)
            psums = [opsp.tile([128, T], F32, tag=f"o{oc}", name=f"ops{oc}")
                     for oc in range(8)]
            n_mm = 2 * 4 * HG
            j = 0
            for gs in range(G):
                for r in range(4 * HG):
                    H = 8 * (r // 4) + 4 * gs + (r % 4)
                    for oc in range(8):
                        nc.tensor.matmul(
                            psums[oc][:], wo_r[:, H, oc, :], asts[gs][:, r, :],
                            start=(j == 0), stop=(j == n_mm - 1))
                    j += 1
            ev = ost.tile([128, 8, T], F32, tag="oev", bufs=1)
            for oc in range(8):
                nc.scalar.copy(ev[:, oc, :], psums[oc][:])
            nc.sync.dma_start(out_v[:, :, ds(ts0, T)], ev[:])

    pctx.close()


def prep_inputs(inputs):
    hs = np.asarray(inputs["hidden_states"], dtype=np.float32)
    pos = np.asarray(inputs["position_ids"]).astype(np.float64)
    Wq = np.asarray(inputs["Wq"], dtype=np.float32)
    Wk = np.asarray(inputs["Wk"], dtype=np.float32)
    Wv = np.asarray(inputs["Wv"], dtype=np.float32)
    Wo = np.asarray(inputs["Wo"], dtype=np.float32)
    aq = np.asarray(inputs["lora_A_q"], dtype=np.float32)
    bq = np.asarray(inputs["lora_B_q"], dtype=np.float32)
    av = np.asarray(inputs["lora_A_v"], dtype=np.float32)
    bv = np.asarray(inputs["lora_B_v"], dtype=np.float32)

    wq_eff = (Wq * SCALE).astype(BF)
    bq_eff = (bq * (LORA_SCALING * SCALE)).astype(BF)
    bv_eff = (bv * LORA_SCALING).astype(BF)
    wk_b, wv_b, wo_b = Wk.astype(BF), Wv.astype(BF), Wo.astype(BF)
    aq_b, av_b = aq.astype(BF), av.astype(BF)

    # RoPE tables per batch: packed [128, S] with cos rows 0-63, sin 64-127
    inv_freq = 1.0 / (10000.0 ** (np.arange(0, D, 2, dtype=np.float64) / D))
    tabs = []
    for b in range(2):
        freqs = np.outer(pos[b], inv_freq)          # [S, 64]
        tabs.append(np.ascontiguousarray(np.concatenate(
            [np.cos(freqs).T, np.sin(freqs).T]).astype(np.float32)))
    hsT = [np.ascontiguousarray(hs[b].T.astype(BF)) for b in range(2)]

    in_maps = []
    for c in range(8):
        b, s = divmod(c, 4)
        in_maps.append({
            "hstq": hsT[b][1024 * s:1024 * (s + 1)],
            "wq": np.ascontiguousarray(wq_eff[:, 1024 * s:1024 * (s + 1)]),
            "wk": np.ascontiguousarray(wk_b[:, 256 * s:256 * (s + 1)]),
            "wv": np.ascontiguousarray(wv_b[:, 256 * s:256 * (s + 1)]),
            "wo": np.ascontiguousarray(wo_b[:, 1024 * s:1024 * (s + 1)]),
            "aq": aq_b, "av": av_b,
            "bq": np.ascontiguousarray(bq_eff[:, 1024 * s:1024 * (s + 1)]),
            "bv": np.ascontiguousarray(bv_eff[:, 256 * s:256 * (s + 1)]),
            "cst": tabs[b],
        })
    return in_maps


def assemble(results):
    out = np.empty((2, S, HID), dtype=np.float32)
    for c in range(8):
        b, r = divmod(c, 4)
        out[b, :, 1024 * r:1024 * (r + 1)] = results[c]["out"].T
    return out


def run_prepped(in_maps, null=False, iters=1):
    nc = build_nc(null=null, iters=iters)
    return run_bass_kernel_spmd(nc, in_maps, list(range(8)), trace=False)


def kernel(**inputs) -> np.ndarray:
    in_maps = prep_inputs(inputs)
    res = run_prepped(in_maps)
    return assemble(res.results)


# revision 27
# speedup vs baseline: 5.7559x; 1.6604x over previous
"""Mistral sliding-window GQA attention + LoRA on 8 trn2 cores.

Sharding: DP2 x TP4. Core c -> batch b=c//4, head-slot s=c%4.
Each core: 8 q heads (2 kv groups of 4), full 2048-token sequence.

bf16 datapath (weights + activations shipped as bf16; PSUM accumulation
fp32). Hidden states are streamed once; q/k/v/LoRA projections for both
kv groups run in a single For_i loop over 512-token chunks. Attention
in transposed layout (S^T tiles [k,q]) with softmax denominators via
ones-matmul and band masks via affine_select; one For_i loop over the
4 heads of each kv group. AllGather(bf16) shares attention outputs
across each 4-core batch group; the output projection is a For_i loop
over 512-token chunks. Hardware loops + batched rearranged DMAs keep
the emitted instruction count (and so NEFF size / load time) small.
"""
import math
from contextlib import ExitStack

import ml_dtypes
import numpy as np

import concourse.bass as bass
import concourse.mybir as mybir
import concourse.tile as tile
from concourse import bacc
from concourse.bass import ds
from concourse.bass_utils import run_bass_kernel_spmd
from concourse.masks import make_identity

F32 = mybir.dt.float32
BF16 = mybir.dt.bfloat16
AF = mybir.ActivationFunctionType
BF = ml_dtypes.bfloat16

HID = 4096
S = 2048
D = 128
WIN = 1024
NHQ = 8          # q heads per core
G = 2            # kv groups per core
HG = 4           # q heads per kv group
T = 512          # token chunk (matmul free dim)
NT = S // T      # 4
NHC = HID // 128  # 32 hidden chunks
LORA_R = 16
SCALE = 1.0 / math.sqrt(D)
LORA_SCALING = 2.0


def ktiles_for(q0):
    return [k0 for k0 in range(0, S, 128) if -384 <= q0 - k0 <= 1024]


_CACHE = {}


def build_nc(null=False, iters=1, upto="full", dump=False):
    key = ("null" if null else "full", iters, upto, dump)
    if key in _CACHE:
        return _CACHE[key]
    nc = bacc.Bacc("TRN2", target_bir_lowering=False, debug=False,
                   num_devices=8)
    d = {}
    for name, shape, dt in [
        ("hstq", [HID // 4, S], BF16), ("wq", [HID, 1024], BF16),
        ("wk", [HID, 256], BF16), ("wv", [HID, 256], BF16),
        ("wo", [HID, 1024], BF16), ("aq", [HID, LORA_R], BF16),
        ("bq", [LORA_R, 1024], BF16), ("av", [HID, LORA_R], BF16),
        ("bv", [LORA_R, 256], BF16), ("cst", [128, S], F32),
    ]:
        d[name] = nc.dram_tensor(name, shape, dt, kind="ExternalInput").ap()
    out = nc.dram_tensor("out", [1024, S], BF16, kind="ExternalOutput").ap()
    if dump:
        for name, shape in [("qtg_d", [128, NHQ, S]), ("ktg_d", [128, G, S]),
                            ("vng_d", [128, G, S]),
                            ("as_d", [NHQ, 128, S])]:
            d[name] = nc.dram_tensor(name, shape, BF16,
                                     kind="ExternalOutput").ap()

    if null:
        _build_null(nc, d, out)
    else:
        _build_body(nc, d, out, iters, upto, dump)
    nc.compile()
    _CACHE[key] = nc
    return nc


def _build_null(nc, d, out):
    with tile.TileContext(nc) as tc:
        with tc.tile_pool(name="sb", bufs=2) as sb:
            t = sb.tile([128, S], BF16)
            nc.gpsimd.memset(t[:], 0.0)
            for i in range(8):
                nc.sync.dma_start(out[128 * i:128 * (i + 1), :], t[:])


def _build_body(nc, d, out, iters=1, upto="full", dump=False):
    with tile.TileContext(nc) as tc, ExitStack() as octx:
        cp = octx.enter_context(tc.tile_pool(name="const", bufs=1))
        dp = octx.enter_context(tc.tile_pool(name="dram", bufs=1, space="DRAM"))

        ident = cp.tile([128, 128], F32)
        make_identity(nc, ident[:])
        ones_b = cp.tile([128, 1], BF16)
        nc.gpsimd.memset(ones_b[:], 1.0)

        attn_spill = dp.tile([NHQ, 128, NT, T], BF16)
        ag = [dp.tile([4 * HG, 128, S], BF16, name=f"ag{g}") for g in range(G)]

        # gather the full hidden-state transpose from per-core quarters
        # (collectives may not read IO tensors -> bounce through a dram tile)
        hst_q = dp.tile([HID // 4, S], BF16, name="hstb")
        nc.sync.dma_start(hst_q[:], d["hstq"][:])
        hst_full = dp.tile([HID, S], BF16, name="hstf")
        nc.gpsimd.collective_compute(
            "AllGather", mybir.AluOpType.bypass,
            replica_groups=[[0, 1, 2, 3], [4, 5, 6, 7]],
            ins=[hst_q.opt()], outs=[hst_full.opt()])
        d = dict(d, hst=hst_full[:])

        for rep in range(iters):
            _one_rep(nc, tc, d, out, rep, ident, ones_b, attn_spill, ag, upto,
                     dump)


def _one_rep(nc, tc, d, out, rep, ident, ones_b, attn_spill, ag, upto="full",
             dump=False):
    pctx = ExitStack()
    ap_ = pctx.enter_context(tc.tile_pool(name=f"act{rep}", bufs=1))

    # ---- attention operands (filled by projection loop) ----
    qtg = ap_.tile([128, NHQ, NT, T], BF16)
    ktg = ap_.tile([128, G, NT, T], BF16)
    vng = ap_.tile([128, G, NT, T], BF16)   # v^T: [k-part, t, d-cols]

    hst_v = d["hst"].rearrange("(c p) (t s) -> p c t s", p=128, t=NT)

    def rope_into(ps, cs, sn, dst, tmp):
        # dst = ps*cos + rotate_half(ps)*sin  (halves along partition dim)
        c1 = tmp.tile([128, T], F32, tag="rpc", bufs=1)
        nc.vector.tensor_mul(c1[0:64, :], ps[0:64, :], cs)
        nc.vector.tensor_mul(c1[64:128, :], ps[64:128, :], cs)
        s1 = tmp.tile([128, T], F32, tag="rps", bufs=1)
        nc.vector.tensor_mul(s1[0:64, :], ps[64:128, :], sn)
        nc.vector.tensor_mul(s1[64:128, :], ps[0:64, :], sn)
        ro = tmp.tile([128, T], BF16, tag="rpo", bufs=2)
        nc.vector.tensor_sub(ro[0:64, :], c1[0:64, :], s1[0:64, :])
        nc.vector.tensor_add(ro[64:128, :], c1[64:128, :], s1[64:128, :])
        nc.vector.tensor_copy(dst, ro[:])

    # =================== projection phase ===================
    with tc.tile_pool(name=f"w{rep}", bufs=1) as wp, \
         tc.tile_pool(name=f"pst{rep}", bufs=1) as pst, \
         tc.tile_pool(name=f"pps{rep}", bufs=1, space="PSUM") as pps:
        # resident weights (single rearranged DMAs)
        wq_r = wp.tile([128, NHC, 1024], BF16)
        nc.sync.dma_start(wq_r[:], d["wq"].rearrange("(c p) n -> p c n", p=128))
        wk_r = wp.tile([128, NHC, 256], BF16)
        nc.sync.dma_start(wk_r[:], d["wk"].rearrange("(c p) n -> p c n", p=128))
        wv_r = wp.tile([128, NHC, 256], BF16)
        nc.sync.dma_start(wv_r[:], d["wv"].rearrange("(c p) n -> p c n", p=128))
        aq_r = wp.tile([128, NHC, LORA_R], BF16)
        nc.sync.dma_start(aq_r[:], d["aq"].rearrange("(c p) r -> p c r", p=128))
        av_r = wp.tile([128, NHC, LORA_R], BF16)
        nc.sync.dma_start(av_r[:], d["av"].rearrange("(c p) r -> p c r", p=128))
        bq_r = wp.tile([LORA_R, 1024], BF16)
        nc.sync.dma_start(bq_r[:], d["bq"][:])
        bv_r = wp.tile([LORA_R, 256], BF16)
        nc.sync.dma_start(bv_r[:], d["bv"][:])
        cst_v = d["cst"].rearrange("p (t s) -> p t s", t=NT)

        with tc.For_i(0, NT, 1, name=f"proj{rep}") as t:
            hst_r = pst.tile([128, NHC, T], BF16, tag="hst", bufs=1)
            nc.sync.dma_start(hst_r[:], hst_v[:, :, ds(t, 1), :])
            cst_c = pst.tile([128, T], F32, tag="cst", bufs=1)
            nc.sync.dma_start(cst_c[:], cst_v[:, ds(t, 1), :])
            cs = cst_c[0:64, :]
            sn = cst_c[64:128, :]
            qps = [pps.tile([128, T], F32, tag=f"q{i}", name=f"qps{i}")
                   for i in range(HG)]
            kps = pps.tile([128, T], F32, tag="k")
            vps = pps.tile([128, T], F32, tag="v")
            lpq = pps.tile([LORA_R, T], F32, tag="lq")
            lpv = pps.tile([LORA_R, T], F32, tag="lv")
            tmq = pst.tile([LORA_R, T], BF16, tag="tmq", bufs=1)
            tmv = pst.tile([LORA_R, T], BF16, tag="tmv", bufs=1)
            for g in range(G):
                for hc in range(NHC):
                    h = hst_r[:, hc, :]
                    for i in range(HG):
                        c0 = 512 * g + 128 * i
                        nc.tensor.matmul(qps[i][:], wq_r[:, hc, c0:c0 + 128],
                                         h, start=(hc == 0), stop=False)
                    nc.tensor.matmul(kps[:], wk_r[:, hc, 128 * g:128 * (g + 1)],
                                     h, start=(hc == 0), stop=(hc == NHC - 1))
                    nc.tensor.matmul(vps[:], wv_r[:, hc, 128 * g:128 * (g + 1)],
                                     h, start=(hc == 0), stop=False)
                    if g == 0:
                        nc.tensor.matmul(lpq[:], aq_r[:, hc, :], h,
                                         start=(hc == 0), stop=(hc == NHC - 1))
                        nc.tensor.matmul(lpv[:], av_r[:, hc, :], h,
                                         start=(hc == 0), stop=(hc == NHC - 1))
                if g == 0:
                    nc.vector.tensor_copy(tmq[:], lpq[:])
                    nc.vector.tensor_copy(tmv[:], lpv[:])
                # LoRA second stage closes the accumulation groups
                for i in range(HG):
                    c0 = 512 * g + 128 * i
                    nc.tensor.matmul(qps[i][:], bq_r[:, c0:c0 + 128], tmq[:],
                                     start=False, stop=True)
                nc.tensor.matmul(vps[:], bv_r[:, 128 * g:128 * (g + 1)],
                                 tmv[:], start=False, stop=True)
                # epilogues: RoPE q/k, transpose v
                for i in range(HG):
                    rope_into(qps[i], cs, sn, qtg[:, g * HG + i, ds(t, 1), :],
                              pst)
                rope_into(kps, cs, sn, ktg[:, g, ds(t, 1), :], pst)
                vev = pst.tile([128, T], F32, tag="vev", bufs=1)
                nc.vector.tensor_copy(vev[:], vps[:])
                for tt in range(4):
                    vtp = pps.tile([128, 128], F32, tag="lq" if tt % 2 == 0
                                   else "lv", name=f"vtp{tt}")
                    nc.tensor.transpose(vtp[:], vev[:, 128 * tt:128 * (tt + 1)],
                                        ident[:])
                    nc.vector.tensor_copy(
                        vng[:, g, ds(t, 1), 128 * tt:128 * (tt + 1)], vtp[:])

    if dump:
        nc.sync.dma_start(d["qtg_d"][:],
                          qtg[:].rearrange("p h t s -> p h (t s)"))
        nc.sync.dma_start(d["ktg_d"][:],
                          ktg[:].rearrange("p g t s -> p g (t s)"))
        nc.sync.dma_start(d["vng_d"][:],
                          vng[:].rearrange("p g t s -> p g (t s)"))
    if upto == "proj":
        pctx.close()
        return

    # wo resident load overlaps the attention phase
    op = pctx.enter_context(tc.tile_pool(name=f"wo{rep}", bufs=1))
    wo_r = op.tile([128, NHC, 8, 128], BF16)
    nc.sync.dma_start(
        wo_r[:], d["wo"].rearrange("(c p) (o q) -> p c o q", p=128, o=8))

    # =================== attention phase ===================
    ktg_f = [ktg[:, g].rearrange("p t s -> p (t s)") for g in range(G)]
    vng_f = [vng[:, g].rearrange("p t s -> p (t s)") for g in range(G)]
    with tc.tile_pool(name=f"am{rep}", bufs=1) as amp, \
         tc.tile_pool(name=f"aps{rep}", bufs=1, space="PSUM") as aps:
        for g in range(G):
            with tc.For_i(g * HG, (g + 1) * HG, 1, name=f"attn{g}_{rep}") as hh:
                for qc in range(NT):
                    q0 = qc * T
                    kts = ktiles_for(q0)
                    avp = aps.tile([128, T], F32, tag="avps", bufs=2)
                    dnp = aps.tile([1, T], F32, tag="dps", bufs=2)
                    last = len(kts) - 1
                    for ki, k0 in enumerate(kts):
                        sps = aps.tile([128, T], F32, tag="sps", bufs=2)
                        nc.tensor.matmul(
                            sps[:], ktg_f[g][:, k0:k0 + 128],
                            qtg[:, ds(hh, 1), qc, :], start=True, stop=True)
                        d0 = q0 - k0
                        at = amp.tile([128, T], BF16, tag="at", bufs=3)
                        nc.scalar.activation(at[:], sps[:], AF.Exp)
                        if d0 - 127 < 0:
                            # zero where (qq - kk + d0) < 0  (causal)
                            nc.gpsimd.affine_select(
                                out=at[:], in_=at[:], pattern=[[1, T]],
                                compare_op=mybir.AluOpType.is_ge,
                                fill=0.0, base=d0, channel_multiplier=-1)
                        if d0 + T - 1 > WIN - 1:
                            # zero where (qq - kk + d0) > WIN-1 (window)
                            nc.gpsimd.affine_select(
                                out=at[:], in_=at[:], pattern=[[-1, T]],
                                compare_op=mybir.AluOpType.is_ge,
                                fill=0.0, base=WIN - 1 - d0,
                                channel_multiplier=1)
                        nc.tensor.matmul(avp[:], vng_f[g][:, k0:k0 + 128],
                                         at[:], start=(ki == 0),
                                         stop=(ki == last))
                        nc.tensor.matmul(dnp[:], ones_b[:], at[:],
                                         start=(ki == 0), stop=(ki == last))
                    rc = amp.tile([1, T], F32, tag="rc", bufs=2)
                    nc.vector.reciprocal(rc[:], dnp[:])
                    bc = amp.tile([128, T], F32, tag="bc", bufs=2)
                    nc.gpsimd.partition_broadcast(bc[:], rc[:])
                    ao = amp.tile([128, T], BF16, tag="ao", bufs=2)
                    nc.vector.tensor_mul(ao[:], avp[:], bc[:])
                    nc.sync.dma_start(attn_spill[ds(hh, 1), :, qc, :], ao[:])
        if dump:
            with tc.tile_pool(name=f"dmp{rep}", bufs=1) as dmp:
                for h in range(NHQ):
                    dt_ = dmp.tile([128, S], BF16, tag="dd", bufs=2)
                    nc.sync.dma_start(
                        dt_[:],
                        attn_spill[h].rearrange("p t s -> p (t s)"))
                    nc.sync.dma_start(d["as_d"][h], dt_[:])
        if upto == "full":
            for g in range(G):
                nc.gpsimd.collective_compute(
                    "AllGather", mybir.AluOpType.bypass,
                    replica_groups=[[0, 1, 2, 3], [4, 5, 6, 7]],
                    ins=[attn_spill[HG * g:HG * (g + 1)].opt()],
                    outs=[ag[g].opt()])

    # =================== output projection ===================
    # global head H lives at ag[(H % 8) // 4][4 * (H // 8) + H % 4]
    ag_v = [ag[g].rearrange("r p s -> p r s") for g in range(G)]
    out_v = out.rearrange("(o p) s -> p o s", p=128)
    with tc.tile_pool(name=f"ost{rep}", bufs=1) as ost, \
         tc.tile_pool(name=f"ops{rep}", bufs=1, space="PSUM") as opsp:
        with tc.For_i(0, S, T, name=f"oproj{rep}") as ts0:
            asts = []
            for gs in range(G):
                ast = ost.tile([128, 4 * HG, T], BF16, tag=f"ast{gs}", bufs=1)
                nc.sync.dma_start(ast[:], ag_v[gs][:, :, ds(ts0, T)])
                asts.append(ast)
            psums = [opsp.tile([128, T], F32, tag=f"o{oc}", name=f"ops{oc}")
                     for oc in range(8)]
            n_mm = 2 * 4 * HG
            j = 0
            for gs in range(G):
                for r in range(4 * HG):
                    H = 8 * (r // 4) + 4 * gs + (r % 4)
                    for oc in range(8):
                        nc.tensor.matmul(
                            psums[oc][:], wo_r[:, H, oc, :], asts[gs][:, r, :],
                            start=(j == 0), stop=(j == n_mm - 1))
                    j += 1
            ev = ost.tile([128, 8, T], BF16, tag="oev", bufs=1)
            for oc in range(8):
                nc.scalar.copy(ev[:, oc, :], psums[oc][:])
            nc.sync.dma_start(out_v[:, :, ds(ts0, T)], ev[:])

    pctx.close()


def prep_inputs(inputs):
    hs = np.asarray(inputs["hidden_states"], dtype=np.float32)
    pos = np.asarray(inputs["position_ids"]).astype(np.float64)
    Wq = np.asarray(inputs["Wq"], dtype=np.float32)
    Wk = np.asarray(inputs["Wk"], dtype=np.float32)
    Wv = np.asarray(inputs["Wv"], dtype=np.float32)
    Wo = np.asarray(inputs["Wo"], dtype=np.float32)
    aq = np.asarray(inputs["lora_A_q"], dtype=np.float32)
    bq = np.asarray(inputs["lora_B_q"], dtype=np.float32)
    av = np.asarray(inputs["lora_A_v"], dtype=np.float32)
    bv = np.asarray(inputs["lora_B_v"], dtype=np.float32)

    wq_eff = (Wq * SCALE).astype(BF)
    bq_eff = (bq * (LORA_SCALING * SCALE)).astype(BF)
    bv_eff = (bv * LORA_SCALING).astype(BF)
    wk_b, wv_b, wo_b = Wk.astype(BF), Wv.astype(BF), Wo.astype(BF)
    aq_b, av_b = aq.astype(BF), av.astype(BF)

    # RoPE tables per batch: packed [128, S] with cos rows 0-63, sin 64-127
    inv_freq = 1.0 / (10000.0 ** (np.arange(0, D, 2, dtype=np.float64) / D))
    tabs = []
    for b in range(2):
        freqs = np.outer(pos[b], inv_freq)          # [S, 64]
        tabs.append(np.ascontiguousarray(np.concatenate(
            [np.cos(freqs).T, np.sin(freqs).T]).astype(np.float32)))
    hsT = [np.ascontiguousarray(hs[b].T.astype(BF)) for b in range(2)]

    in_maps = []
    for c in range(8):
        b, s = divmod(c, 4)
        in_maps.append({
            "hstq": hsT[b][1024 * s:1024 * (s + 1)],
            "wq": np.ascontiguousarray(wq_eff[:, 1024 * s:1024 * (s + 1)]),
            "wk": np.ascontiguousarray(wk_b[:, 256 * s:256 * (s + 1)]),
            "wv": np.ascontiguousarray(wv_b[:, 256 * s:256 * (s + 1)]),
            "wo": np.ascontiguousarray(wo_b[:, 1024 * s:1024 * (s + 1)]),
            "aq": aq_b, "av": av_b,
            "bq": np.ascontiguousarray(bq_eff[:, 1024 * s:1024 * (s + 1)]),
            "bv": np.ascontiguousarray(bv_eff[:, 256 * s:256 * (s + 1)]),
            "cst": tabs[b],
        })
    return in_maps


def assemble(results):
    out = np.empty((2, S, HID), dtype=np.float32)
    for c in range(8):
        b, r = divmod(c, 4)
        out[b, :, 1024 * r:1024 * (r + 1)] = \
            np.asarray(results[c]["out"]).astype(np.float32).T
    return out


def run_prepped(in_maps, null=False, iters=1):
    nc = build_nc(null=null, iters=iters)
    return run_bass_kernel_spmd(nc, in_maps, list(range(8)), trace=False)


def kernel(**inputs) -> np.ndarray:
    in_maps = prep_inputs(inputs)
    res = run_prepped(in_maps)
    return assemble(res.results)


# revision 34
# speedup vs baseline: 6.4451x; 1.1197x over previous
"""Mistral sliding-window GQA attention + LoRA on 8 trn2 cores.

Sharding: DP2 x TP4. Core c -> batch b=c//4, head-slot s=c%4.
Each core: 8 q heads (2 kv groups of 4), full 2048-token sequence.

bf16 datapath (weights + activations shipped as bf16; PSUM accumulation
fp32). Hidden states are streamed once; q/k/v/LoRA projections for both
kv groups run in a single For_i loop over 512-token chunks. Attention
in transposed layout (S^T tiles [k,q]) with softmax denominators via
ones-matmul and band masks via affine_select; one For_i loop over the
4 heads of each kv group. AllGather(bf16) shares attention outputs
across each 4-core batch group; the output projection is a For_i loop
over 512-token chunks. Hardware loops + batched rearranged DMAs keep
the emitted instruction count (and so NEFF size / load time) small.
"""
import math
from contextlib import ExitStack

import ml_dtypes
import numpy as np

import concourse.bass as bass
import concourse.mybir as mybir
import concourse.tile as tile
from concourse import bacc
from concourse.bass import ds
from concourse.bass_utils import run_bass_kernel_spmd
from concourse.masks import make_identity

F32 = mybir.dt.float32
BF16 = mybir.dt.bfloat16
AF = mybir.ActivationFunctionType
BF = ml_dtypes.bfloat16

HID = 4096
S = 2048
D = 128
WIN = 1024
NHQ = 8          # q heads per core
G = 2            # kv groups per core
HG = 4           # q heads per kv group
T = 512          # token chunk (matmul free dim)
NT = S // T      # 4
NHC = HID // 128  # 32 hidden chunks
LORA_R = 16
SCALE = 1.0 / math.sqrt(D)
LORA_SCALING = 2.0


def ktiles_for(q0):
    return [k0 for k0 in range(0, S, 128) if -384 <= q0 - k0 <= 1024]


_CACHE = {}


def build_nc(null=False, iters=1, upto="full", dump=False):
    key = ("null" if null else "full", iters, upto, dump)
    if key in _CACHE:
        return _CACHE[key]
    nc = bacc.Bacc("TRN2", target_bir_lowering=False, debug=False,
                   num_devices=8)
    d = {}
    for name, shape, dt in [
        ("hstq", [HID // 4, S], BF16), ("wq", [HID, 1024], BF16),
        ("wk", [HID, 256], BF16), ("wv", [HID, 256], BF16),
        ("wo", [HID, 1024], BF16), ("aqv", [HID, 3 * LORA_R], BF16),
        ("bq", [LORA_R, 1024], BF16),
        ("bv", [LORA_R, 256], BF16), ("cst", [128, S], F32),
    ]:
        d[name] = nc.dram_tensor(name, shape, dt, kind="ExternalInput").ap()
    out = nc.dram_tensor("out", [1024, S], BF16, kind="ExternalOutput").ap()
    if dump:
        for name, shape in [("qtg_d", [128, NHQ, S]), ("ktg_d", [128, G, S]),
                            ("vng_d", [128, G, S]),
                            ("as_d", [NHQ, 128, S])]:
            d[name] = nc.dram_tensor(name, shape, BF16,
                                     kind="ExternalOutput").ap()

    if null:
        _build_null(nc, d, out)
    else:
        _build_body(nc, d, out, iters, upto, dump)
    nc.compile()
    _CACHE[key] = nc
    return nc


def _build_null(nc, d, out):
    with tile.TileContext(nc) as tc:
        with tc.tile_pool(name="sb", bufs=2) as sb:
            t = sb.tile([128, S], BF16)
            nc.gpsimd.memset(t[:], 0.0)
            for i in range(8):
                nc.sync.dma_start(out[128 * i:128 * (i + 1), :], t[:])


def _build_body(nc, d, out, iters=1, upto="full", dump=False):
    with tile.TileContext(nc) as tc, ExitStack() as octx:
        cp = octx.enter_context(tc.tile_pool(name="const", bufs=1))
        dp = octx.enter_context(tc.tile_pool(name="dram", bufs=1, space="DRAM"))

        ident = cp.tile([128, 128], F32)
        make_identity(nc, ident[:])
        ones_b = cp.tile([128, 1], BF16)
        nc.gpsimd.memset(ones_b[:], 1.0)

        attn_spill = dp.tile([NHQ, 128, NT, T], BF16)
        ag = [dp.tile([4 * HG, 128, S], BF16, name=f"ag{g}") for g in range(G)]

        # gather the full hidden-state transpose from per-core quarters
        # (collectives may not read IO tensors -> bounce through a dram tile)
        hst_q = dp.tile([HID // 4, S], BF16, name="hstb")
        nc.sync.dma_start(hst_q[:], d["hstq"][:])
        hst_full = dp.tile([HID, S], BF16, name="hstf")
        nc.gpsimd.collective_compute(
            "AllGather", mybir.AluOpType.bypass,
            replica_groups=[[0, 1, 2, 3], [4, 5, 6, 7]],
            ins=[hst_q.opt()], outs=[hst_full.opt()])
        d = dict(d, hst=hst_full[:])

        for rep in range(iters):
            _one_rep(nc, tc, d, out, rep, ident, ones_b, attn_spill, ag, upto,
                     dump)


def _one_rep(nc, tc, d, out, rep, ident, ones_b, attn_spill, ag, upto="full",
             dump=False):
    pctx = ExitStack()
    ap_ = pctx.enter_context(tc.tile_pool(name=f"act{rep}", bufs=1))

    # ---- attention operands (filled by projection loop) ----
    qtg = ap_.tile([128, NHQ, NT, T], BF16)
    ktg = ap_.tile([128, G, NT, T], BF16)
    vng = ap_.tile([128, G, NT, T], BF16)   # v^T: [k-part, t, d-cols]

    hst_v = d["hst"].rearrange("(c p) (t s) -> p c t s", p=128, t=NT)

    def rope_into(ps, cs, sn, dst, tmp):
        # dst = ps*cos + rotate_half(ps)*sin  (halves along partition dim)
        c1 = tmp.tile([128, T], F32, tag="rpc", bufs=1)
        nc.vector.tensor_mul(c1[0:64, :], ps[0:64, :], cs)
        nc.vector.tensor_mul(c1[64:128, :], ps[64:128, :], cs)
        s1 = tmp.tile([128, T], F32, tag="rps", bufs=1)
        nc.vector.tensor_mul(s1[0:64, :], ps[64:128, :], sn)
        nc.vector.tensor_mul(s1[64:128, :], ps[0:64, :], sn)
        ro = tmp.tile([128, T], BF16, tag="rpo", bufs=2)
        nc.vector.tensor_sub(ro[0:64, :], c1[0:64, :], s1[0:64, :])
        nc.vector.tensor_add(ro[64:128, :], c1[64:128, :], s1[64:128, :])
        nc.vector.tensor_copy(dst, ro[:])

    # =================== projection phase ===================
    with tc.tile_pool(name=f"w{rep}", bufs=1) as wp, \
         tc.tile_pool(name=f"pst{rep}", bufs=1) as pst, \
         tc.tile_pool(name=f"pps{rep}", bufs=1, space="PSUM") as pps:
        # resident weights (single rearranged DMAs)
        wq_r = wp.tile([128, NHC, 1024], BF16)
        nc.sync.dma_start(wq_r[:], d["wq"].rearrange("(c p) n -> p c n", p=128))
        wk_r = wp.tile([128, NHC, 256], BF16)
        nc.sync.dma_start(wk_r[:], d["wk"].rearrange("(c p) n -> p c n", p=128))
        wv_r = wp.tile([128, NHC, 256], BF16)
        nc.sync.dma_start(wv_r[:], d["wv"].rearrange("(c p) n -> p c n", p=128))
        aqv_r = wp.tile([128, NHC, 3 * LORA_R], BF16)
        nc.sync.dma_start(aqv_r[:],
                          d["aqv"].rearrange("(c p) r -> p c r", p=128))
        bq_r = wp.tile([LORA_R, 1024], BF16)
        nc.sync.dma_start(bq_r[:], d["bq"][:])
        bv_r = wp.tile([LORA_R, 256], BF16)
        nc.sync.dma_start(bv_r[:], d["bv"][:])
        cst_v = d["cst"].rearrange("p (t s) -> p t s", t=NT)

        with tc.For_i(0, NT, 1, name=f"proj{rep}") as t:
            hst_r = pst.tile([128, NHC, T], BF16, tag="hst", bufs=1)
            nc.sync.dma_start(hst_r[:], hst_v[:, :, ds(t, 1), :])
            cst_c = pst.tile([128, T], F32, tag="cst", bufs=1)
            nc.sync.dma_start(cst_c[:], cst_v[:, ds(t, 1), :])
            cs = cst_c[0:64, :]
            sn = cst_c[64:128, :]
            qps = [pps.tile([128, T], F32, tag=f"q{i}", name=f"qps{i}")
                   for i in range(HG)]
            kps = pps.tile([128, T], F32, tag="k")
            vps = pps.tile([128, T], F32, tag="v")
            lqv = pps.tile([3 * LORA_R, T], F32, tag="lq")
            tmq = pst.tile([LORA_R, T], BF16, tag="tmq", bufs=1)
            tmv = pst.tile([LORA_R, T], BF16, tag="tmv", bufs=1)
            for g in range(G):
                for hc in range(NHC):
                    h = hst_r[:, hc, :]
                    for i in range(HG):
                        c0 = 512 * g + 128 * i
                        nc.tensor.matmul(qps[i][:], wq_r[:, hc, c0:c0 + 128],
                                         h, start=(hc == 0), stop=False)
                    nc.tensor.matmul(kps[:], wk_r[:, hc, 128 * g:128 * (g + 1)],
                                     h, start=(hc == 0), stop=(hc == NHC - 1))
                    nc.tensor.matmul(vps[:], wv_r[:, hc, 128 * g:128 * (g + 1)],
                                     h, start=(hc == 0), stop=False)
                    if g == 0:
                        nc.tensor.matmul(lqv[:], aqv_r[:, hc, :], h,
                                         start=(hc == 0), stop=(hc == NHC - 1))
                if g == 0:
                    nc.vector.tensor_copy(tmq[:], lqv[0:LORA_R, :])
                    nc.vector.tensor_copy(tmv[:], lqv[2 * LORA_R:3 * LORA_R, :])
                # LoRA second stage closes the accumulation groups
                for i in range(HG):
                    c0 = 512 * g + 128 * i
                    nc.tensor.matmul(qps[i][:], bq_r[:, c0:c0 + 128], tmq[:],
                                     start=False, stop=True)
                nc.tensor.matmul(vps[:], bv_r[:, 128 * g:128 * (g + 1)],
                                 tmv[:], start=False, stop=True)
                # epilogues: RoPE q/k, transpose v
                for i in range(HG):
                    rope_into(qps[i], cs, sn, qtg[:, g * HG + i, ds(t, 1), :],
                              pst)
                rope_into(kps, cs, sn, ktg[:, g, ds(t, 1), :], pst)
                vev = pst.tile([128, T], F32, tag="vev", bufs=1)
                nc.vector.tensor_copy(vev[:], vps[:])
                for tt in range(4):
                    vtp = pps.tile([128, 128], F32, tag="lq" if tt % 2 == 0
                                   else "lv", name=f"vtp{tt}")
                    nc.tensor.transpose(vtp[:], vev[:, 128 * tt:128 * (tt + 1)],
                                        ident[:])
                    nc.vector.tensor_copy(
                        vng[:, g, ds(t, 1), 128 * tt:128 * (tt + 1)], vtp[:])

    if dump:
        nc.sync.dma_start(d["qtg_d"][:],
                          qtg[:].rearrange("p h t s -> p h (t s)"))
        nc.sync.dma_start(d["ktg_d"][:],
                          ktg[:].rearrange("p g t s -> p g (t s)"))
        nc.sync.dma_start(d["vng_d"][:],
                          vng[:].rearrange("p g t s -> p g (t s)"))
    if upto == "proj":
        pctx.close()
        return

    # wo resident load overlaps the attention phase
    op = pctx.enter_context(tc.tile_pool(name=f"wo{rep}", bufs=1))
    wo_r = op.tile([128, NHC, 8, 128], BF16)
    nc.sync.dma_start(
        wo_r[:], d["wo"].rearrange("(c p) (o q) -> p c o q", p=128, o=8))

    # =================== attention phase ===================
    ktg_f = [ktg[:, g].rearrange("p t s -> p (t s)") for g in range(G)]
    vng_f = [vng[:, g].rearrange("p t s -> p (t s)") for g in range(G)]
    with tc.tile_pool(name=f"am{rep}", bufs=1) as amp, \
         tc.tile_pool(name=f"aps{rep}", bufs=1, space="PSUM") as aps:
        for g in range(G):
            with tc.For_i(g * HG, (g + 1) * HG, 1, name=f"attn{g}_{rep}") as hh:
                for qc in range(NT):
                    q0 = qc * T
                    kts = ktiles_for(q0)
                    avp = aps.tile([128, T], F32, tag="avps", bufs=2)
                    dnp = aps.tile([1, T], F32, tag="dps", bufs=2)
                    last = len(kts) - 1
                    for ki, k0 in enumerate(kts):
                        sps = aps.tile([128, T], F32, tag="sps", bufs=2)
                        nc.tensor.matmul(
                            sps[:], ktg_f[g][:, k0:k0 + 128],
                            qtg[:, ds(hh, 1), qc, :], start=True, stop=True)
                        d0 = q0 - k0
                        at = amp.tile([128, T], BF16, tag="at", bufs=3)
                        nc.scalar.activation(at[:], sps[:], AF.Exp)
                        if d0 - 127 < 0:
                            # zero where (qq - kk + d0) < 0  (causal)
                            nc.gpsimd.affine_select(
                                out=at[:], in_=at[:], pattern=[[1, T]],
                                compare_op=mybir.AluOpType.is_ge,
                                fill=0.0, base=d0, channel_multiplier=-1)
                        if d0 + T - 1 > WIN - 1:
                            # zero where (qq - kk + d0) > WIN-1 (window)
                            nc.gpsimd.affine_select(
                                out=at[:], in_=at[:], pattern=[[-1, T]],
                                compare_op=mybir.AluOpType.is_ge,
                                fill=0.0, base=WIN - 1 - d0,
                                channel_multiplier=1)
                        nc.tensor.matmul(avp[:], vng_f[g][:, k0:k0 + 128],
                                         at[:], start=(ki == 0),
                                         stop=(ki == last))
                        nc.tensor.matmul(dnp[:], ones_b[:], at[:],
                                         start=(ki == 0), stop=(ki == last))
                    rc = amp.tile([1, T], F32, tag="rc", bufs=2)
                    nc.vector.reciprocal(rc[:], dnp[:])
                    bc = amp.tile([128, T], F32, tag="bc", bufs=2)
                    nc.gpsimd.partition_broadcast(bc[:], rc[:])
                    if qc == 0:
                        aob = amp.tile([128, NT, T], BF16, tag="ao", bufs=2)
                    nc.vector.tensor_mul(aob[:, qc, :], avp[:], bc[:])
                nc.sync.dma_start(attn_spill[ds(hh, 1), :, :, :], aob[:])
        if dump:
            with tc.tile_pool(name=f"dmp{rep}", bufs=1) as dmp:
                for h in range(NHQ):
                    dt_ = dmp.tile([128, S], BF16, tag="dd", bufs=2)
                    nc.sync.dma_start(
                        dt_[:],
                        attn_spill[h].rearrange("p t s -> p (t s)"))
                    nc.sync.dma_start(d["as_d"][h], dt_[:])
        if upto == "full":
            for g in range(G):
                nc.gpsimd.collective_compute(
                    "AllGather", mybir.AluOpType.bypass,
                    replica_groups=[[0, 1, 2, 3], [4, 5, 6, 7]],
                    ins=[attn_spill[HG * g:HG * (g + 1)].opt()],
                    outs=[ag[g].opt()])

    # =================== output projection ===================
    # global head H lives at ag[(H % 8) // 4][4 * (H // 8) + H % 4]
    ag_v = [ag[g].rearrange("r p s -> p r s") for g in range(G)]
    out_v = out.rearrange("(o p) s -> p o s", p=128)
    with tc.tile_pool(name=f"ost{rep}", bufs=1) as ost, \
         tc.tile_pool(name=f"ops{rep}", bufs=1, space="PSUM") as opsp:
        with tc.For_i(0, S, T, name=f"oproj{rep}") as ts0:
            asts = []
            for gs in range(G):
                ast = ost.tile([128, 4 * HG, T], BF16, tag=f"ast{gs}", bufs=1)
                nc.sync.dma_start(ast[:], ag_v[gs][:, :, ds(ts0, T)])
                asts.append(ast)
            psums = [opsp.tile([128, T], F32, tag=f"o{oc}", name=f"ops{oc}")
                     for oc in range(8)]
            n_mm = 2 * 4 * HG
            j = 0
            for gs in range(G):
                for r in range(4 * HG):
                    H = 8 * (r // 4) + 4 * gs + (r % 4)
                    for oc in range(8):
                        nc.tensor.matmul(
                            psums[oc][:], wo_r[:, H, oc, :], asts[gs][:, r, :],
                            start=(j == 0), stop=(j == n_mm - 1))
                    j += 1
            ev = ost.tile([128, 8, T], BF16, tag="oev", bufs=1)
            for oc in range(8):
                nc.scalar.copy(ev[:, oc, :], psums[oc][:])
            nc.sync.dma_start(out_v[:, :, ds(ts0, T)], ev[:])

    pctx.close()


def prep_inputs(inputs):
    hs = np.asarray(inputs["hidden_states"], dtype=np.float32)
    pos = np.asarray(inputs["position_ids"]).astype(np.float64)
    Wq = np.asarray(inputs["Wq"], dtype=np.float32)
    Wk = np.asarray(inputs["Wk"], dtype=np.float32)
    Wv = np.asarray(inputs["Wv"], dtype=np.float32)
    Wo = np.asarray(inputs["Wo"], dtype=np.float32)
    aq = np.asarray(inputs["lora_A_q"], dtype=np.float32)
    bq = np.asarray(inputs["lora_B_q"], dtype=np.float32)
    av = np.asarray(inputs["lora_A_v"], dtype=np.float32)
    bv = np.asarray(inputs["lora_B_v"], dtype=np.float32)

    wq_eff = (Wq * SCALE).astype(BF)
    bq_eff = (bq * (LORA_SCALING * SCALE)).astype(BF)
    bv_eff = (bv * LORA_SCALING).astype(BF)
    wk_b, wv_b, wo_b = Wk.astype(BF), Wv.astype(BF), Wo.astype(BF)
    aqv_b = np.ascontiguousarray(
        np.concatenate([aq, np.zeros_like(aq), av], axis=1).astype(BF))

    # RoPE tables per batch: packed [128, S] with cos rows 0-63, sin 64-127
    inv_freq = 1.0 / (10000.0 ** (np.arange(0, D, 2, dtype=np.float64) / D))
    tabs = []
    for b in range(2):
        freqs = np.outer(pos[b], inv_freq)          # [S, 64]
        tabs.append(np.ascontiguousarray(np.concatenate(
            [np.cos(freqs).T, np.sin(freqs).T]).astype(np.float32)))
    hsT = [np.ascontiguousarray(hs[b].T.astype(BF)) for b in range(2)]

    in_maps = []
    for c in range(8):
        b, s = divmod(c, 4)
        in_maps.append({
            "hstq": hsT[b][1024 * s:1024 * (s + 1)],
            "wq": np.ascontiguousarray(wq_eff[:, 1024 * s:1024 * (s + 1)]),
            "wk": np.ascontiguousarray(wk_b[:, 256 * s:256 * (s + 1)]),
            "wv": np.ascontiguousarray(wv_b[:, 256 * s:256 * (s + 1)]),
            "wo": np.ascontiguousarray(wo_b[:, 1024 * s:1024 * (s + 1)]),
            "aqv": aqv_b,
            "bq": np.ascontiguousarray(bq_eff[:, 1024 * s:1024 * (s + 1)]),
            "bv": np.ascontiguousarray(bv_eff[:, 256 * s:256 * (s + 1)]),
            "cst": tabs[b],
        })
    return in_maps


def assemble(results):
    out = np.empty((2, S, HID), dtype=np.float32)
    for c in range(8):
        b, r = divmod(c, 4)
        out[b, :, 1024 * r:1024 * (r + 1)] = \
            np.asarray(results[c]["out"]).astype(np.float32).T
    return out


def run_prepped(in_maps, null=False, iters=1):
    nc = build_nc(null=null, iters=iters)
    return run_bass_kernel_spmd(nc, in_maps, list(range(8)), trace=False)


def kernel(**inputs) -> np.ndarray:
    in_maps = prep_inputs(inputs)
    res = run_prepped(in_maps)
    return assemble(res.results)
